# revision 1
# baseline (speedup 1.0000x reference)
import sys

sys.path.insert(0, "/opt/trn_rl_repo")

import hashlib

import numpy as np

import concourse.bass as bass
import concourse.mybir as mybir
import concourse.tile as tile
from concourse.library_config import mlp
from concourse.masks import make_identity
from concourse.vector_clock import ScopedClock

dt = mybir.dt
AF = mybir.ActivationFunctionType
ALU = mybir.AluOpType

N_NODES = 100000
F_IN = 128
N_CLASSES = 40
C_PAD = 64
NCORES = 8
NSH = 12500
NT = 98
NSHP = NT * 128  # 12544
QROWS = 2 * NSHP  # 25088 rows per int16-indexable quarter
NQ = 4
KCAP = 8  # max 128-idx chunks per gather call (bounds SBUF tile sizes)


class PatchedTileContext(tile.TileContext):
    # walrus CoreV3 codegen accepts at most 1 sem wait on most instruction
    # structs; spread the final-drain waits over 1-wait nops.
    def _drain_and_barrier(self, tick_clock, wait_clock):
        collector = self.nc.sync.nop(nofuse=True)
        wait_clock.add_sem_waits(
            collector.ins, ScopedClock({None: tick_clock.global_clock})
        )
        si = collector.ins.sync_info
        waits = list(si.on_wait) if si and si.on_wait else []
        if len(waits) > 1:
            si.on_wait = waits[:1]
            for w in waits[1:]:
                extra = self.nc.sync.nop(nofuse=True)
                extra.ins.sync_info = mybir.SyncInfo(on_wait=[w], on_update=[])
        self.nc.sync.drain()
        self.nc.all_engine_barrier()
        assert self.sems is not None
        popped = self.nc._tile_sem_poison_stack.pop()
        assert popped is self._sem_poison
        self.nc.clear_and_free_semaphores(list(self.sems.allocated().values()))
        self.nc.all_engine_barrier()


def _split_excess_waits(nc, max_waits=1):
    # Same walrus limit for ordinary instructions: move excess sem waits onto
    # single-wait carrier instructions on the same engine, inserted just
    # before (per-engine order makes the stall equivalent).
    cnt = 0
    for f in nc.m.functions:
        for bb in f.blocks:
            insns = bb.instructions
            i = 0
            while i < len(insns):
                ins = insns[i]
                si = getattr(ins, "sync_info", None)
                waits = list(si.on_wait) if si is not None and si.on_wait else []
                if len(waits) > max_waits:
                    si.on_wait = waits[:1]
                    for w in waits[1:]:
                        if ins.engine == mybir.EngineType.Pool:
                            nop = mybir.InstEventSemaphore(
                                name=f"waitsplit_{cnt}", ins=[], outs=[]
                            )
                        else:
                            nop = mybir.InstNoOp(
                                name=f"waitsplit_{cnt}", ins=[], outs=[]
                            )
                        cnt += 1
                        nop.engine = ins.engine
                        nop.sync_info = mybir.SyncInfo(on_wait=[w], on_update=[])
                        insns.insert(i, nop)
                        i += 1
                i += 1
    return cnt


def _preprocess(edge_index):
    src = np.asarray(edge_index[0], dtype=np.int64)
    dst = np.asarray(edge_index[1], dtype=np.int64)
    deg = np.bincount(dst, minlength=N_NODES).astype(np.float32) + 1.0
    dinv = (1.0 / np.sqrt(deg)).astype(np.float32)

    core_of = dst // NSH
    per_core = []
    counts = np.zeros((NCORES, NT * NQ), np.int64)
    for c in range(NCORES):
        m = core_of == c
        es = src[m]
        ed = dst[m] - c * NSH
        t = ed >> 7
        slot = ed & 127
        pr = (es // NSH) * NSHP + (es % NSH)
        q = pr // QROWS
        lidx = pr % QROWS
        key = t * NQ + q
        order = np.argsort(key, kind="stable")
        key = key[order]
        lidx = lidx[order]
        slot = slot[order]
        cnt = np.bincount(key, minlength=NT * NQ)
        counts[c] = cnt
        per_core.append((key, lidx, slot, cnt))

    K = np.ceil(counts / 128.0).astype(np.int64).max(axis=0)  # [NT*NQ]
    kmax = int(K.max())
    nchunk = int(K.sum())
    tot = nchunk * 128
    chunk_off = np.concatenate([[0], np.cumsum(K)]).astype(np.int64)

    idx_ws, dst_ws, dinv_cs = [], [], []
    for c in range(NCORES):
        key, lidx, slot, cnt = per_core[c]
        starts = np.cumsum(cnt) - cnt
        j = np.arange(len(key)) - starts[key]
        pos = chunk_off[key] * 128 + j
        idx_flat = np.zeros(tot, np.int16)
        dst_flat = np.full(tot, 999.0, np.float32)
        idx_flat[pos] = lidx.astype(np.int16)
        dst_flat[pos] = slot.astype(np.float32)
        # [128, tot//16]: 16-row wrap replicated 8x (one copy per gpsimd core)
        idx_ws.append(
            np.ascontiguousarray(np.tile(idx_flat.reshape(tot // 16, 16).T, (8, 1)))
        )
        dst_ws.append(np.ascontiguousarray(dst_flat.reshape(nchunk, 128).T))
        dloc = np.zeros(NSHP, np.float32)
        dloc[:NSH] = dinv[c * NSH : (c + 1) * NSH]
        dinv_cs.append(np.ascontiguousarray(dloc.reshape(NT, 128).T))

    nch_t = (
        chunk_off[np.arange(1, NT + 1) * NQ] - chunk_off[np.arange(NT) * NQ]
    )
    nch_max = max(int(nch_t.max()), 1)
    iota = np.tile(np.arange(128, dtype=np.float32)[None, :], (128, nch_max))
    iota = np.ascontiguousarray(iota.reshape(128, nch_max, 128))
    return dict(
        K=K,
        kmax=kmax,
        nchunk=nchunk,
        tot=tot,
        chunk_off=chunk_off,
        nch_max=nch_max,
        idx_ws=idx_ws,
        dst_ws=dst_ws,
        dinv_cs=dinv_cs,
        iota=iota,
    )


def _build(meta, ablate=()):
    K = meta["K"]
    kmax = meta["kmax"]
    nchunk = meta["nchunk"]
    tot = meta["tot"]
    chunk_off = meta["chunk_off"]
    nch_max = meta["nch_max"]
    KG = max(min(KCAP, kmax), 1)

    nc = bass.Bass(num_devices=NCORES, num_swdge_queues=4)
    xT_in = nc.dram_tensor("xT_s", [F_IN, NSHP], dt.float32, kind="ExternalInput")
    w1_in = nc.dram_tensor("w1", [F_IN, F_IN], dt.float32, kind="ExternalInput")
    w2_in = nc.dram_tensor("w2", [F_IN, 128], dt.float32, kind="ExternalInput")
    dinv_in = nc.dram_tensor("dinv_c", [128, NT], dt.float32, kind="ExternalInput")
    idx_in = nc.dram_tensor("idx_w", [128, tot // 16], dt.int16, kind="ExternalInput")
    dst_in = nc.dram_tensor("dst_w", [128, nchunk], dt.float32, kind="ExternalInput")
    iota_in = nc.dram_tensor(
        "iota_r", [128, nch_max, 128], dt.float32, kind="ExternalInput"
    )
    out_t = nc.dram_tensor("out_s", [NSHP, N_CLASSES], dt.float32, kind="ExternalOutput")

    with PatchedTileContext(nc) as tc:
        with (
            tc.tile_pool(name="sbuf", bufs=1) as pool,
            tc.tile_pool(name="psum", bufs=1, space="PSUM") as psum,
            tc.tile_pool(name="dram", bufs=1, space="DRAM") as dram,
        ):
            w1_t = pool.tile([F_IN, F_IN], dt.float32)
            w2_t = pool.tile([F_IN, 128], dt.float32)
            dinv_t = pool.tile([128, NT], dt.float32)
            idx_t = pool.tile([128, tot // 16], dt.int16)
            dst_t = pool.tile([128, nchunk], dt.float32)
            iota_t = pool.tile([128, nch_max, 128], dt.float32)
            ident = pool.tile([128, 128], dt.float32)
            h_all = pool.tile([128, NT, F_IN], dt.float32)
            z_all = pool.tile([128, NT, N_CLASSES], dt.float32)
            zs_all = pool.tile([128, NT, N_CLASSES], dt.float32)
            znorm = pool.tile([128, NT, N_CLASSES], dt.float32)
            nc.gpsimd.load_library(mlp)
            for d_ap, s_ap in [
                (w1_t, w1_in),
                (w2_t, w2_in),
                (dinv_t, dinv_in),
                (idx_t, idx_in),
                (dst_t, dst_in),
                (iota_t, iota_in),
            ]:
                nc.sync.dma_start(d_ap[:], s_ap[:])
            make_identity(nc, ident[:])

            bounce1 = dram.tile([NSHP, F_IN], dt.bfloat16)
            table1 = dram.tile(
                [NCORES * NSHP, F_IN], dt.bfloat16, addr_space="Shared"
            )
            bounce2 = dram.tile([NSHP, 128], dt.bfloat16)
            table2 = dram.tile(
                [NCORES * NSHP, 128], dt.bfloat16, addr_space="Shared"
            )

            # Phase A: h~ = dinv * (x @ W1), publish bf16 copy for AllGather
            _sidA, _ = nc.enter_named_scope("phaseA", False)
            for t in range(NT):
                xT = pool.tile([128, 128], dt.float32, name="xT", bufs=3)
                nc.sync.dma_start(xT[:], xT_in[:, t * 128 : (t + 1) * 128])
                mm = psum.tile([128, F_IN], dt.float32, name="mm", bufs=2)
                nc.tensor.matmul(mm[:], lhsT=xT[:], rhs=w1_t[:], start=True, stop=True)
                nc.scalar.activation(
                    h_all[:, t, :], mm[:], AF.Copy, bias=0.0, scale=dinv_t[:, t : t + 1]
                )
                h16 = pool.tile([128, F_IN], dt.bfloat16, name="h16", bufs=3)
                nc.scalar.copy(h16[:], h_all[:, t, :])
                nc.sync.dma_start(bounce1[t * 128 : (t + 1) * 128, :], h16[:])
            nc.leave_named_scope("phaseA", _sidA, False)

            _sidG1, _ = nc.enter_named_scope("ag1", False)
            if "cc" not in ablate:
                nc.gpsimd.collective_compute(
                    "AllGather",
                    ALU.bypass,
                    replica_groups=[list(range(NCORES))],
                    ins=[bounce1.opt()],
                    outs=[table1.opt()],
                )
            nc.leave_named_scope("ag1", _sidG1, False)

            # gpsimd registers are scarce: one per distinct idx count, reused
            reg_cache = {}

            def nreg(v):
                if v not in reg_cache:
                    reg_cache[v] = nc.gpsimd.to_reg(v)
                return reg_cache[v]

            # Phase B: aggregate layer 1, then transform for layer 2
            _sidB, _ = nc.enter_named_scope("phaseB", False)
            for t in range(NT):
                off0 = int(chunk_off[t * NQ])
                nch = int(chunk_off[(t + 1) * NQ]) - off0
                acc = psum.tile([128, 128], dt.float32, name="acc", bufs=2)
                if nch > 0:
                    sel = pool.tile(
                        [128, nch_max, 128], dt.bfloat16, name="sel", bufs=2
                    )
                    nc.vector.tensor_tensor(
                        out=sel[:, 0:nch, :],
                        in0=dst_t[:, off0 : off0 + nch].to_broadcast([128, nch, 128]),
                        in1=iota_t[:, 0:nch, :],
                        op=ALU.is_equal,
                    )
                done = 0
                for q in range(NQ):
                    kb = int(K[t * NQ + q])
                    off = int(chunk_off[t * NQ + q])
                    for p0 in range(0, kb, KCAP):
                        kp = min(KCAP, kb - p0)
                        o = off + p0
                        g = pool.tile(
                            [128, KG, F_IN], dt.bfloat16, name=f"g{q}", bufs=2
                        )
                        if "gather" not in ablate:
                            nc.gpsimd.dma_gather(
                                g[:, 0:kp, :],
                                table1[q * QROWS : (q + 1) * QROWS, :],
                                idx_t[:, o * 8 : (o + kp) * 8],
                                num_idxs=kp * 128,
                                num_idxs_reg=nreg(kp * 128),
                                elem_size=F_IN,
                                queue_num=q,
                            )
                        loc = o - off0
                        for cch in range(kp):
                            if "pe" in ablate:
                                break
                            nc.tensor.matmul(
                                acc[:],
                                lhsT=sel[:, loc + cch, :],
                                rhs=g[:, cch, :],
                                start=(done == 0),
                                stop=(done == nch - 1),
                            )
                            done += 1
                agg = pool.tile([128, 128], dt.float32, name="agg", bufs=2)
                if nch > 0:
                    nc.vector.tensor_tensor(
                        out=agg[:], in0=acc[:], in1=h_all[:, t, :], op=ALU.add
                    )
                else:
                    nc.vector.tensor_copy(agg[:], h_all[:, t, :])
                h1 = pool.tile([128, 128], dt.float32, name="h1", bufs=2)
                nc.scalar.activation(
                    h1[:], agg[:], AF.Relu, bias=0.0, scale=dinv_t[:, t : t + 1]
                )
                tp = psum.tile([128, 128], dt.float32, name="tp", bufs=2)
                nc.tensor.transpose(tp[:], h1[:], ident[:])
                hT = pool.tile([128, 128], dt.float32, name="hT", bufs=2)
                nc.scalar.copy(hT[:], tp[:])
                mm = psum.tile([128, 128], dt.float32, name="mm", bufs=2)
                nc.tensor.matmul(mm[:], lhsT=hT[:], rhs=w2_t[:], start=True, stop=True)
                nc.scalar.activation(
                    z_all[:, t, :],
                    mm[:, 0:N_CLASSES],
                    AF.Copy,
                    bias=0.0,
                    scale=dinv_t[:, t : t + 1],
                )
                z16 = pool.tile([128, 128], dt.bfloat16, name="z16", bufs=3)
                nc.scalar.activation(
                    z16[:], mm[:], AF.Copy, bias=0.0, scale=dinv_t[:, t : t + 1]
                )
                nc.sync.dma_start(bounce2[t * 128 : (t + 1) * 128, :], z16[:])
            nc.leave_named_scope("phaseB", _sidB, False)

            _sidG2, _ = nc.enter_named_scope("ag2", False)
            if "cc" not in ablate:
                nc.gpsimd.collective_compute(
                    "AllGather",
                    ALU.bypass,
                    replica_groups=[list(range(NCORES))],
                    ins=[bounce2.opt()],
                    outs=[table2.opt()],
                )
            nc.leave_named_scope("ag2", _sidG2, False)

            # Phase C: aggregate layer 2, batched log_softmax, write out
            _sidC, _ = nc.enter_named_scope("phaseC", False)
            for t in range(NT):
                off0 = int(chunk_off[t * NQ])
                nch = int(chunk_off[(t + 1) * NQ]) - off0
                acc = psum.tile([128, 128], dt.float32, name="acc", bufs=2)
                if nch > 0:
                    sel = pool.tile(
                        [128, nch_max, 128], dt.bfloat16, name="sel", bufs=2
                    )
                    nc.vector.tensor_tensor(
                        out=sel[:, 0:nch, :],
                        in0=dst_t[:, off0 : off0 + nch].to_broadcast([128, nch, 128]),
                        in1=iota_t[:, 0:nch, :],
                        op=ALU.is_equal,
                    )
                done = 0
                for q in range(NQ):
                    kb = int(K[t * NQ + q])
                    off = int(chunk_off[t * NQ + q])
                    for p0 in range(0, kb, KCAP):
                        kp = min(KCAP, kb - p0)
                        ofs = off + p0
                        g = pool.tile(
                            [128, KG, 128], dt.bfloat16, name=f"g{q}", bufs=2
                        )
                        if "gather" not in ablate:
                            nc.gpsimd.dma_gather(
                                g[:, 0:kp, :],
                                table2[q * QROWS : (q + 1) * QROWS, :],
                                idx_t[:, ofs * 8 : (ofs + kp) * 8],
                                num_idxs=kp * 128,
                                num_idxs_reg=nreg(kp * 128),
                                elem_size=128,
                                queue_num=q,
                            )
                        loc = ofs - off0
                        for cch in range(kp):
                            if "pe" in ablate:
                                break
                            nc.tensor.matmul(
                                acc[:],
                                lhsT=sel[:, loc + cch, :],
                                rhs=g[:, cch, :],
                                start=(done == 0),
                                stop=(done == nch - 1),
                            )
                            done += 1
                if nch > 0:
                    nc.vector.tensor_tensor(
                        out=zs_all[:, t, :],
                        in0=acc[:, 0:N_CLASSES],
                        in1=z_all[:, t, :],
                        op=ALU.add,
                    )
                else:
                    nc.vector.tensor_copy(zs_all[:, t, :], z_all[:, t, :])
            nc.vector.tensor_tensor(
                out=znorm[:],
                in0=zs_all[:],
                in1=dinv_t[:, :].to_broadcast([128, NT, N_CLASSES]),
                op=ALU.mult,
            )
            mx = pool.tile([128, NT, 1], dt.float32, name="mx")
            nc.vector.tensor_reduce(mx[:], znorm[:], mybir.AxisListType.X, ALU.max)
            nc.vector.tensor_tensor(
                out=zs_all[:],
                in0=znorm[:],
                in1=mx[:, :, 0].to_broadcast([128, NT, N_CLASSES]),
                op=ALU.subtract,
            )
            nc.scalar.activation(znorm[:], zs_all[:], AF.Exp, bias=0.0, scale=1.0)
            sm = pool.tile([128, NT, 1], dt.float32, name="sm")
            nc.vector.tensor_reduce(sm[:], znorm[:], mybir.AxisListType.X, ALU.add)
            ls = pool.tile([128, NT, 1], dt.float32, name="ls")
            nc.scalar.activation(ls[:], sm[:], AF.Ln, bias=0.0, scale=1.0)
            nc.vector.tensor_tensor(
                out=znorm[:],
                in0=zs_all[:],
                in1=ls[:, :, 0].to_broadcast([128, NT, N_CLASSES]),
                op=ALU.subtract,
            )
            for t in range(NT):
                nc.sync.dma_start(
                    out_t[t * 128 : (t + 1) * 128, :], znorm[:, t, :]
                )
            nc.leave_named_scope("phaseC", _sidC, False)

    _split_excess_waits(nc)
    mybir.codegen_inst_isa_subclasses(nc)
    return nc


def _make_runner(nc):
    import jax
    from jax.sharding import Mesh, PartitionSpec

    try:
        from jax.experimental.shard_map import shard_map
    except ImportError:
        from jax.shard_map import shard_map

    from concourse.bass2jax import (
        _bass_exec_p,
        install_neuronx_cc_hook,
        partition_id_tensor,
    )

    install_neuronx_cc_hook()
    assert nc.dbg_addr is None
    partition_name = nc.partition_id_tensor.name if nc.partition_id_tensor else None

    in_names, out_names, out_avals = [], [], []
    for alloc in nc.m.functions[0].allocations:
        if not isinstance(alloc, mybir.MemoryLocationSet):
            continue
        name = alloc.memorylocations[0].name
        if alloc.kind == "ExternalInput":
            if name != partition_name:
                in_names.append(name)
        elif alloc.kind == "ExternalOutput":
            out_names.append(name)
            shape = tuple(alloc.tensor_shape)
            dtype = mybir.dt.np(alloc.dtype)
            out_avals.append(jax.core.ShapedArray(shape, dtype))
    n_params = len(in_names)
    n_outs = len(out_avals)
    all_names = in_names + out_names
    if partition_name is not None:
        all_names = all_names + [partition_name]
    donate = tuple(range(n_params, n_params + n_outs))

    def _body(*args):
        operands = list(args)
        if partition_name is not None:
            operands.append(partition_id_tensor())
        outs = _bass_exec_p.bind(
            *operands,
            out_avals=tuple(out_avals),
            in_names=tuple(all_names),
            out_names=tuple(out_names),
            lowering_input_output_aliases=(),
            sim_require_finite=True,
            sim_require_nnan=True,
            nc=nc,
        )
        return tuple(outs)

    devices = jax.devices()[:NCORES]
    mesh = Mesh(np.asarray(devices), ("core",))
    in_specs = (PartitionSpec("core"),) * (n_params + n_outs)
    out_specs = (PartitionSpec("core"),) * n_outs
    sharded = jax.jit(
        shard_map(
            _body, mesh=mesh, in_specs=in_specs, out_specs=out_specs, check_rep=False
        ),
        donate_argnums=donate,
        keep_unused=True,
    )

    state = {"dev_in": None, "dev_key": None}

    def run(in_maps):
        per_core = [[np.asarray(m[name]) for name in in_names] for m in in_maps]
        concat_in = [
            np.concatenate([per_core[c][i] for c in range(NCORES)], axis=0)
            for i in range(n_params)
        ]
        hkey = hashlib.sha1()
        for a in concat_in:
            hkey.update(a.tobytes())
        hkey = hkey.hexdigest()
        if state["dev_key"] != hkey:
            from jax.sharding import NamedSharding

            state["dev_in"] = [
                jax.device_put(a, NamedSharding(mesh, PartitionSpec("core")))
                for a in concat_in
            ]
            state["dev_key"] = hkey
        concat_zeros = [
            np.zeros((NCORES * a.shape[0], *a.shape[1:]), a.dtype) for a in out_avals
        ]
        out_arrs = sharded(*state["dev_in"], *concat_zeros)
        jax.block_until_ready(out_arrs)
        return [
            [
                np.asarray(out_arrs[i]).reshape(NCORES, *out_avals[i].shape)[c]
                for i in range(n_outs)
            ]
            for c in range(NCORES)
        ]

    run.sharded = sharded
    run.state = state
    run.mesh = mesh
    run.out_avals = out_avals
    run.body = _body
    run.n_params = n_params
    return run


_CACHE = {}


def kernel(**inputs):
    x = np.asarray(inputs["x"], np.float32)
    ei = np.asarray(inputs["edge_index"])
    W1 = np.asarray(inputs["W1"], np.float32)
    W2 = np.asarray(inputs["W2"], np.float32)
    b1 = np.asarray(inputs["b1"], np.float32)
    b2 = np.asarray(inputs["b2"], np.float32)
    assert not b1.any() and not b2.any(), "nonzero biases not supported"

    key = hashlib.sha1(ei.tobytes()).hexdigest()
    st = _CACHE.get(key)
    if st is None:
        meta = _preprocess(ei)
        nc = _build(meta)
        runner = _make_runner(nc)
        st = {"meta": meta, "runner": runner}
        _CACHE.clear()
        _CACHE[key] = st
    meta = st["meta"]

    w2p = np.zeros((F_IN, 128), np.float32)
    w2p[:, :N_CLASSES] = W2
    in_maps = []
    for c in range(NCORES):
        xs = np.zeros((NSHP, F_IN), np.float32)
        xs[:NSH] = x[c * NSH : (c + 1) * NSH]
        in_maps.append(
            {
                "xT_s": np.ascontiguousarray(xs.T),
                "w1": W1,
                "w2": w2p,
                "dinv_c": meta["dinv_cs"][c],
                "idx_w": meta["idx_ws"][c],
                "dst_w": meta["dst_ws"][c],
                "iota_r": meta["iota"],
            }
        )
    outs = st["runner"](in_maps)
    return np.concatenate([outs[c][0][:NSH] for c in range(NCORES)], axis=0)

